# revision 1
# baseline (speedup 1.0000x reference)
"""MiniBatchDiscrimination Trainium2 kernel (8-core SPMD).

Reference computation:
    m = (x @ T).reshape(B, OUT_F, NUM_K)            # B=256, OUT_F=128, NUM_K=16
    dists = |m[None,:,:,:] - m[:,None,:,:]|         # [B, B, OUT_F, NUM_K]
    out = sum_i exp(-sum_k dists) - 1               # [B, OUT_F]
    return concat([x, out], axis=-1)                # [B, 640]

Strategy (per core, identical SPMD program, per-core data):
  * Each core owns JB=32 output rows (j). Full m is computed on every core
    (replicated GEMM, cheap) so no collectives are needed.
  * m is stored in SBUF as [partition p=(f8,k), free n=(i,f_o)] with
    f = f_o*8 + f8, p = f8*16 + k. Then T's columns c = f*16+k satisfy
    c = f_o*128 + p, i.e. each f_o corresponds to a contiguous 128-column
    block of T -> the GEMM producing this layout is 16 plain matmuls.
  * Pairwise pass per i: DVE bf16 tensor_sub (2x mode) of m_shard vs
    m_all[:,i] broadcast over j; abs on ACT (Abs activation) or DVE
    (tensor_scalar abs_max, 4x mode) -- split tunable; then the k-sum is a
    TensorE matmul with a block-diagonal ones [128,8] writing dist rows for
    16 consecutive i into one PSUM bank [128=(i_sub,f8), 512=(j,f_o)].
  * exp(-dist): ACT Exp with scale=-1 from PSUM -> bf16 SBUF.
  * sum over i: TensorE matmuls with a second ones pattern accumulating in
    PSUM across all 16 blocks -> [8, 512] = out[f8, (j, f_o)].
  * Host unshards: reshape to [32,128] per core, concat with x.
"""

import os
import numpy as np

import concourse.bass as bass
import concourse.tile as tile
from concourse import bacc, mybir

BF16 = mybir.dt.bfloat16
FP32 = mybir.dt.float32
NPBF16 = np.dtype(mybir.dt.np(BF16))

B = 256
IN_F = 512
OUT_F = 128
NUM_K = 16
N_CORES = 8
JB = B // N_CORES          # 32 j-rows owned per core
F8 = 8                     # f8 = f % 8   (partition group)
FO = OUT_F // F8           # 16 f_o values (free dim)
KC = IN_F // 128           # 4 contraction chunks for the GEMM
NBLK = B // 16             # 16 i-blocks of 16
SUBI = 8                   # i's per DVE sub instruction
# per 8-i sub-batch, how many i's of the abs go to the scalar engine
# (the rest run on DVE as 4x-mode sign-bit stripping)
ACT_ABS_N = int(os.environ.get("ACT_ABS_N", "5"))


def build_nc():
    nc = bacc.Bacc(name="minibatch_discrim")

    # host-prearranged [p, c, i] so each partition's DMA line is contiguous;
    # columns B..B+JB repeat this core's own j-columns so one FD=288 matmul
    # produces m_all and m_sh together (identical values -> exact diagonal).
    xT_d = nc.dram_tensor("xT", [128, KC, B + JB], BF16, kind="ExternalInput")
    # T pre-permuted on host to [fo][p][c][128 cols] so each fo block is
    # one contiguous 128KB DMA (1KB per partition line) that unblocks that
    # fo's GEMM immediately.
    T_d = nc.dram_tensor("T_w", [FO, 128, KC, 128], BF16, kind="ExternalInput")
    onk_d = nc.dram_tensor("ones_k", [128, 8 * 64], BF16, kind="ExternalInput")
    ona_d = nc.dram_tensor("ones_acc", [128, F8], BF16, kind="ExternalInput")
    out_d = nc.dram_tensor("out_pair", [F8, JB * FO], FP32, kind="ExternalOutput")

    with tile.TileContext(nc) as tc:
        with (
            tc.tile_pool(name="const", bufs=1) as constp,
            tc.tile_pool(name="mm", bufs=1) as mmp,
            tc.tile_pool(name="gpsum", bufs=4, space=bass.MemorySpace.PSUM) as gps,
            tc.tile_pool(name="dpsum", bufs=3, space=bass.MemorySpace.PSUM) as dps,
            tc.tile_pool(name="apsum", bufs=1, space=bass.MemorySpace.PSUM) as aps,
            tc.tile_pool(name="work", bufs=3) as wp,
            tc.tile_pool(name="expp", bufs=3) as ep,
        ):
            # ---- constants / inputs to SBUF ----
            zero_b = constp.tile([128, 1], FP32)
            nc.gpsimd.memset(zero_b[:], 0.0)
            neg1_b = constp.tile([128, 1], FP32)
            nc.gpsimd.memset(neg1_b[:], -1.0)

            # ones_k[:, q8, (q, f8)] = 1 iff q == q8 and p//16 == f8.
            # The k-reduce matmul for i_sub targets the 64-partition slice at
            # offset (isub//8)*64 using pattern q8 = isub%8: its 8 target rows
            # get sum_k, the other 56 rows of the slice accumulate += 0.
            ones_k = constp.tile([128, 8, 64], BF16)
            nc.sync.dma_start(ones_k[:], onk_d.rearrange("p (s q) -> p s q", q=64))
            ones_a = constp.tile([128, F8], BF16)
            nc.sync.dma_start(ones_a[:], ona_d[:])

            # warm the ACT exp/abs table while DMAs run
            warm = constp.tile([128, 1], FP32)
            nc.scalar.activation(
                warm[:], zero_b[:], mybir.ActivationFunctionType.Exp, bias=zero_b[:]
            )

            # xT as [p, c, i]  (contraction chunk c)
            xT_sb = constp.tile([128, KC, B + JB], BF16)
            nc.sync.dma_start(xT_sb[:], xT_d[:])
            # T per-fo tiles; 16 independent DMAs so fo-GEMMs start as soon
            # as their slice lands.
            T_tiles = []
            for fo in range(FO):
                tt = constp.tile([128, KC, 128], BF16, tag=f"T{fo}")
                nc.sync.dma_start(tt[:], T_d[fo])
                T_tiles.append(tt)

            # ---- GEMM: m_all [p=(f8,k), (i, f_o)], m_sh [p, (j, f_o)] ----
            m_all = mmp.tile([128, B, FO], BF16)
            m_sh = mmp.tile([128, JB, FO], BF16)
            for fo in range(FO):
                pm = gps.tile([128, B + JB], FP32, tag="gemm_full")
                for c in range(KC):
                    nc.tensor.matmul(
                        pm[:],
                        T_tiles[fo][:, c, :],
                        xT_sb[:, c, :],
                        start=(c == 0),
                        stop=(c == KC - 1),
                    )
                nc.scalar.copy(m_all[:, :, fo], pm[:, :B])
                nc.vector.tensor_copy(m_sh[:, :, fo], pm[:, B:])

            # ---- main pairwise loop ----
            acc = aps.tile([F8, JB * FO], FP32)  # sum over i of exp(-dist)
            sub_idx = 0
            for blk in range(NBLK):
                pd = dps.tile([128, JB * FO], FP32, tag="dist")
                for h in range(16 // SUBI):
                    i0 = blk * 16 + h * SUBI
                    diff = wp.tile([128, SUBI, JB, FO], BF16, tag="diff")
                    nc.vector.tensor_sub(
                        diff[:],
                        m_sh[:, None, :, :].broadcast_to([128, SUBI, JB, FO]),
                        m_all[:, i0:i0 + SUBI, None, :].broadcast_to(
                            [128, SUBI, JB, FO]
                        ),
                    )
                    ad = wp.tile([128, SUBI, JB, FO], BF16, tag="absd")
                    na = ACT_ABS_N
                    if na > 0:
                        nc.scalar.activation(
                            ad[:, :na], diff[:, :na],
                            mybir.ActivationFunctionType.Abs, bias=zero_b[:],
                        )
                    if na < SUBI:
                        # |x| on DVE at 4x mode: strip the bf16 sign bit
                        nc.vector.tensor_scalar(
                            ad[:, na:].bitcast(mybir.dt.uint16),
                            diff[:, na:].bitcast(mybir.dt.uint16),
                            0x7FFF, None, op0=mybir.AluOpType.bitwise_and,
                        )
                    sub_idx += 1
                    for s in range(SUBI):
                        isub = h * SUBI + s
                        g, q = isub // 8, isub % 8
                        nc.tensor.matmul(
                            pd[g * 64:(g + 1) * 64, :],
                            ones_k[:, q, :],
                            ad[:, s, :, :],
                            start=(q == 0),
                            stop=(q == 7),
                        )
                et = ep.tile([128, JB * FO], BF16, tag="expt")
                nc.scalar.activation(
                    et[:], pd[:],
                    mybir.ActivationFunctionType.Exp, bias=zero_b[:], scale=-1.0,
                )
                nc.tensor.matmul(
                    acc[:],
                    ones_a[:],
                    et[:],
                    start=(blk == 0),
                    stop=(blk == NBLK - 1),
                    skip_group_check=True,
                )

            # ---- tail: subtract 1, store ----
            fin = mmp.tile([F8, JB * FO], FP32)
            nc.vector.tensor_scalar_add(fin[:], acc[:], -1.0)
            nc.sync.dma_start(out_d[:], fin[:])

    nc.finalize()
    return nc


def make_in_maps(x: np.ndarray, T: np.ndarray):
    # xT_h[p, c, i] = x[i, c*128+p]
    xT_h = np.ascontiguousarray(
        x.T.astype(NPBF16).reshape(KC, 128, B).transpose(1, 0, 2)
    )
    T_b = np.ascontiguousarray(T).astype(NPBF16)           # [512, 2048]

    p = np.arange(128)[:, None]
    r = np.arange(F8)[None, :]
    ones_a = np.ascontiguousarray((p % 8 == r).astype(NPBF16))    # [128,8]
    # ones_k[p, q8, q] = 1 iff q == q8*8 + p//16  (q in 0..63)
    q = np.arange(64)[None, None, :]
    s = np.arange(8)[None, :, None]
    ones_k = (q == s * 8 + p[:, :, None] // 16).astype(NPBF16)
    ones_k = np.ascontiguousarray(ones_k.reshape(128, 8 * 64))

    # T_w host-permuted to [fo, p, c, n]: T_perm[fo, p, c, n] = T[c*128+p, fo*128+n]
    T_perm = np.ascontiguousarray(
        T_b.reshape(KC, 128, FO, 128).transpose(2, 1, 0, 3)
    )

    in_maps = []
    for c in range(N_CORES):
        xTc = np.ascontiguousarray(np.concatenate(
            [xT_h, xT_h[:, :, c * JB:(c + 1) * JB]], axis=2
        ))
        in_maps.append({
            "xT": xTc,
            "T_w": T_perm,
            "ones_k": ones_k,
            "ones_acc": ones_a,
        })
    return in_maps


def assemble(x: np.ndarray, pair_parts) -> np.ndarray:
    """pair_parts: list of [8, JB*FO] fp32 per core -> full [B, IN_F+OUT_F]."""
    out = np.empty((B, IN_F + OUT_F), np.float32)
    out[:, :IN_F] = x
    for c, fp in enumerate(pair_parts):
        # fp[f8, j*FO + fo] -> out[c*JB + j, IN_F + fo*8 + f8]
        blk = fp.reshape(F8, JB, FO).transpose(1, 2, 0).reshape(JB, OUT_F)
        out[c * JB:(c + 1) * JB, IN_F:] = blk
    return out


_NC_CACHE = None


def kernel(x: np.ndarray, T: np.ndarray) -> np.ndarray:
    global _NC_CACHE
    from concourse import bass_utils

    if _NC_CACHE is None:
        _NC_CACHE = build_nc()
    nc = _NC_CACHE
    in_maps = make_in_maps(np.asarray(x, np.float32), np.asarray(T, np.float32))
    res = bass_utils.run_bass_kernel_spmd(nc, in_maps, core_ids=list(range(N_CORES)))
    parts = [r["out_pair"].astype(np.float32) for r in res.results]
    return assemble(np.asarray(x, np.float32), parts)



# revision 12
# speedup vs baseline: 1.3720x; 1.3720x over previous
"""MiniBatchDiscrimination Trainium2 kernel (8-core SPMD, symmetric pairs).

Reference computation:
    m = (x @ T).reshape(B, OUT_F, NUM_K)            # B=256, OUT_F=128, NUM_K=16
    dists = |m[None,:,:,:] - m[:,None,:,:]|         # [B, B, OUT_F, NUM_K]
    out = sum_i exp(-sum_k dists) - 1               # [B, OUT_F]
    return concat([x, out], axis=-1)                # [B, 640]

Strategy:
  * The pairwise matrix is symmetric: dist(i,j) = dist(j,i).  Partition the
    256 rows into 16 strips of 16; the 136 unordered strip pairs (120
    off-diagonal + 16 diagonal) are covered by 8 edge-disjoint trails of 17
    edges each (a Walecki Hamiltonian-path decomposition of K16 plus one
    loop per vertex, two loops inserted per path).  Core c walks trail c:
    18 strip "slots", 17 units; unit u is the 16x16 pair block
    (slot[u], slot[u+1]).  Every ordered pair is thus computed exactly once
    globally (a block and its transpose are the same computation; both
    attributions are extracted as an i-sum over partitions and a j-sum over
    the free axis).  The host adds the per-unit partial sums.
  * GEMM: x/T quantized to fp8e4, DoubleRow matmuls (2 contraction rows per
    partition, 0.5 cyc/col).  m kept in bf16 [p=(f8,k), col, fo] with
    f = fo*8+f8, p = f8*16+k, 288 columns = 18 slots x 16 rows.
  * Per unit: DVE bf16 tensor_sub (2x) -> abs split between ACT (Abs
    activation, fp8 out), GpSimd (abs_max, fp8 out) and DVE (sign-bit
    strip, bf16) -> k-sum on TensorE: fp8 DoubleRow matmuls (2 i's per
    instruction) for the fp8 part, plain bf16 matmuls for the rest, into a
    [128=(i,f8), 256=(j,fo)] PSUM tile -> ACT Exp(scale=-1) -> bf16 et ->
    i-sum matmul (ones) into batched PSUM windows + j-sum via DVE
    tensor_reduce over the free j axis.
  * Host: scatter-add the i-sum/j-sum partials per the trail map, -1.0,
    concat with x.
"""

import os
import numpy as np

import concourse.bass as bass
import concourse.tile as tile
from concourse import bacc, mybir

BF16 = mybir.dt.bfloat16
FP32 = mybir.dt.float32
FP8 = mybir.dt.float8e4
U16 = mybir.dt.uint16
NPBF16 = np.dtype(mybir.dt.np(BF16))
NPFP8 = np.dtype(mybir.dt.np(FP8))
DR = mybir.MatmulPerfMode.DoubleRow

B = 256
IN_F = 512
OUT_F = 128
NUM_K = 16
N_CORES = 8
STRIP = 16                 # batch rows per strip
NSLOT = 18                 # strip slots per core (17 units)
NUNIT = 17
NCOL = NSLOT * STRIP       # 288 m columns per core
F8 = 8
FO = OUT_F // F8           # 16

# per-unit abs split over the 16 i's: ACT (fp8 out), GpSimd (fp8 out),
# DVE sign-strip (bf16).  N_ACT + N_GP must be even (DoubleRow pairs).
N_ACT = int(os.environ.get("N_ACT", "10"))
N_GP = int(os.environ.get("N_GP", "0"))
N_DVE = 16 - N_ACT - N_GP
N_FP8 = N_ACT + N_GP
assert N_FP8 % 2 == 0 and N_DVE >= 0

# Walecki trails: 8 edge-disjoint trails x 18 slots covering K16 + 16 loops.
SLOTS = [
    [0, 0, 1, 1, 15, 2, 14, 3, 13, 4, 12, 5, 11, 6, 10, 7, 9, 8],
    [1, 2, 2, 0, 3, 3, 15, 4, 14, 5, 13, 6, 12, 7, 11, 8, 10, 9],
    [2, 3, 1, 4, 4, 0, 5, 5, 15, 6, 14, 7, 13, 8, 12, 9, 11, 10],
    [3, 4, 2, 5, 1, 6, 6, 0, 7, 7, 15, 8, 14, 9, 13, 10, 12, 11],
    [4, 5, 3, 6, 2, 7, 1, 8, 8, 0, 9, 9, 15, 10, 14, 11, 13, 12],
    [5, 6, 4, 7, 3, 8, 2, 9, 1, 10, 10, 0, 11, 11, 15, 12, 14, 13],
    [6, 7, 5, 8, 4, 9, 3, 10, 2, 11, 1, 12, 12, 0, 13, 13, 15, 14],
    [7, 8, 6, 9, 5, 10, 4, 11, 3, 12, 2, 13, 1, 14, 14, 0, 15, 15],
]


def build_nc():
    nc = bacc.Bacc(name="minibatch_discrim_sym")

    # x pre-gathered per core to the 288 slot columns, fp8, DoubleRow layout:
    # xT8[p, c2, t, col] = x_perm[col, c2*256 + t*128 + p]
    xT_d = nc.dram_tensor("xT8", [128, 2, 2, NCOL], FP8, kind="ExternalInput")
    # T8[fo, p, c2, t, n] = T[c2*256 + t*128 + p, fo*128 + n]
    T_d = nc.dram_tensor("T8", [FO, 128, 2, 2, 128], FP8, kind="ExternalInput")
    # DoubleRow k-sum weights (full 128 rows, one variant per i-pair):
    # ones8dr[v, p, t, r] = 1 iff r == 16*v + t*8 + p//16
    on8_d = nc.dram_tensor("ones8dr", [8, 128, 2 * 128], FP8, kind="ExternalInput")
    # plain k-sum weights: ones16[q, p, r] = 1 iff r == q*8 + p//16
    on16_d = nc.dram_tensor("ones16", [4, 128, 32], BF16, kind="ExternalInput")
    # i-sum weights: onesA[q, p, r] = 1 iff r == q*8 + p%8
    onA_d = nc.dram_tensor("onesA", [4, 128, 32], BF16, kind="ExternalInput")
    # outputs: i-sum partials (2 PSUM batches of 16 window-slots) + j-sums
    outi_d = nc.dram_tensor("out_i", [128, 2, STRIP * FO], FP32, kind="ExternalOutput")
    outj_d = nc.dram_tensor("out_j", [128, NUNIT, FO], FP32, kind="ExternalOutput")

    with tile.TileContext(nc) as tc:
        with (
            tc.tile_pool(name="const", bufs=1) as constp,
            tc.tile_pool(name="mm", bufs=1) as mmp,
            tc.tile_pool(name="gpsum", bufs=2, space=bass.MemorySpace.PSUM) as gps,
            tc.tile_pool(name="dpsum", bufs=3, space=bass.MemorySpace.PSUM) as dps,
            tc.tile_pool(name="apsum", bufs=1, space=bass.MemorySpace.PSUM) as aps,
            tc.tile_pool(name="work", bufs=3) as wp,
            tc.tile_pool(name="expp", bufs=3) as ep,
        ):
            # ---- constants / inputs to SBUF ----
            zero_b = constp.tile([128, 1], FP32)
            nc.gpsimd.memset(zero_b[:], 0.0)

            ones8 = constp.tile([128, 8, 2, 128], FP8)
            nc.sync.dma_start(ones8[:], on8_d.rearrange("v p (t r) -> p v t r", r=128))
            ones16 = constp.tile([128, 4, 32], BF16)
            nc.sync.dma_start(ones16[:], on16_d.rearrange("q p r -> p q r"))
            onesA = constp.tile([128, 4, 32], BF16)
            nc.sync.dma_start(onesA[:], onA_d.rearrange("q p r -> p q r"))

            # warm the ACT table (Abs+Exp share one set) while DMAs run
            warm = constp.tile([128, 1], FP32)
            nc.scalar.activation(
                warm[:], zero_b[:], mybir.ActivationFunctionType.Abs, bias=zero_b[:]
            )
            nc.scalar.activation(
                warm[:], zero_b[:], mybir.ActivationFunctionType.Exp, bias=zero_b[:]
            )

            xT_sb = constp.tile([128, 2, 2, NCOL], FP8)
            nc.sync.dma_start(xT_sb[:], xT_d[:])
            T_tiles = []
            for fo in range(FO):
                tt = constp.tile([128, 2, 2, 128], FP8, tag=f"T{fo}")
                nc.sync.dma_start(tt[:], T_d[fo])
                T_tiles.append(tt)

            # ---- GEMM (fp8 DoubleRow): m[p=(f8,k), col, fo] bf16 ----
            m_sb = mmp.tile([128, NCOL, FO], BF16)
            for fo in range(FO):
                pm = gps.tile([128, NCOL], FP32, tag="gemm")
                for c2 in range(2):
                    nc.tensor.matmul(
                        pm[:],
                        T_tiles[fo][:, c2],
                        xT_sb[:, c2],
                        start=(c2 == 0),
                        stop=(c2 == 1),
                        perf_mode=DR,
                    )
                if fo % 2 == 0:
                    nc.scalar.copy(m_sb[:, :, fo], pm[:])
                else:
                    nc.vector.tensor_copy(m_sb[:, :, fo], pm[:])

            # ---- unit loop ----
            outj_sb = mmp.tile([128, NUNIT, FO], FP32)
            outi_ps0 = aps.tile([128, STRIP * FO], FP32, tag="acc0")
            outi_ps1 = aps.tile([128, STRIP * FO], FP32, tag="acc1")
            outi_ps = [outi_ps0, outi_ps1]
            outi_sb = mmp.tile([128, 2, STRIP * FO], FP32)

            for u in range(NUNIT):
                iL, jR = u * STRIP, (u + 1) * STRIP

                # diff[p, i, j, fo] = m[p, iL+i, fo] - m[p, jR+j, fo]
                diff = wp.tile([128, STRIP, STRIP, FO], BF16, tag="diff")
                nc.vector.tensor_sub(
                    diff[:],
                    m_sb[:, iL:jR, None, :].broadcast_to([128, STRIP, STRIP, FO]),
                    m_sb[:, None, jR:jR + STRIP, :].broadcast_to(
                        [128, STRIP, STRIP, FO]
                    ),
                )

                # abs: ACT/GpSimd -> fp8, DVE sign-strip -> bf16
                ad8 = wp.tile([128, N_FP8, STRIP, FO], FP8, tag="ad8")
                if N_ACT > 0:
                    nc.scalar.activation(
                        ad8[:, :N_ACT], diff[:, :N_ACT],
                        mybir.ActivationFunctionType.Abs, bias=zero_b[:],
                    )
                if N_GP > 0:
                    nc.gpsimd.tensor_scalar(
                        ad8[:, N_ACT:N_FP8], diff[:, N_ACT:N_FP8],
                        0.0, None, op0=mybir.AluOpType.abs_max,
                    )
                if N_DVE > 0:
                    ad16 = wp.tile([128, N_DVE, STRIP, FO], BF16, tag="ad16")
                    nc.vector.tensor_scalar(
                        ad16[:].bitcast(U16),
                        diff[:, N_FP8:].bitcast(U16),
                        0x7FFF, None, op0=mybir.AluOpType.bitwise_and,
                    )

                # k-sum into pd[p=(i,f8), (j,fo)].  DoubleRow matmuls must
                # target dst partition 0, so each i-pair uses a full-width
                # lhsT variant (zero rows outside its 16-row block); the
                # bf16 matmuls write 32-row windows.
                pd = dps.tile([128, STRIP, FO], FP32, tag="dist")
                n_ops = N_FP8 // 2 + N_DVE
                k_ = 0
                for p2 in range(N_FP8 // 2):
                    nc.tensor.matmul(
                        pd[:], ones8[:, p2], ad8[:, 2 * p2:2 * p2 + 2],
                        start=(k_ == 0), stop=(k_ == n_ops - 1),
                        perf_mode=DR, tile_position=(0, 0),
                        skip_group_check=True,
                    )
                    k_ += 1
                for i0 in range(N_FP8, 16):
                    w = i0 // 4
                    nc.tensor.matmul(
                        pd[32 * w:32 * (w + 1)],
                        ones16[:, i0 % 4], ad16[:, i0 - N_FP8],
                        start=(k_ == 0), stop=(k_ == n_ops - 1),
                        tile_position=(0, 32 * w),
                        skip_group_check=True,
                    )
                    k_ += 1

                # exp(-dist) -> bf16
                et = ep.tile([128, STRIP, FO], BF16, tag="expt")
                nc.scalar.activation(
                    et[:], pd[:],
                    mybir.ActivationFunctionType.Exp, bias=zero_b[:], scale=-1.0,
                )

                # i-sum: out[rs*8+f8, (j,fo)] per 32-row window, 4 units/window
                bw, w, rs = u // 16, (u % 16) // 4, u % 4
                nc.tensor.matmul(
                    outi_ps[bw][32 * w:32 * (w + 1), :],
                    onesA[:, rs],
                    et[:],
                    start=(rs == 0 or u == 16),
                    stop=(rs == 3 or u == 16),
                    skip_group_check=True,
                    tile_position=(0, 32 * w),
                )

                # j-sum over the free j axis -> [p=(i,f8), fo]
                nc.vector.tensor_reduce(
                    outj_sb[:, u, :],
                    et[:].rearrange("p j f -> p f j"),
                    mybir.AxisListType.X,
                    mybir.AluOpType.add,
                )

                if u == 15:
                    nc.scalar.copy(outi_sb[:, 0], outi_ps[0][:])
                if u == 16:
                    nc.scalar.copy(outi_sb[:32, 1], outi_ps[1][:32])

            nc.sync.dma_start(outi_d[:], outi_sb[:])
            nc.sync.dma_start(outj_d[:], outj_sb[:])

    nc.finalize()
    return nc


def make_in_maps(x: np.ndarray, T: np.ndarray):
    x8 = x.astype(NPFP8)
    T8f = T.astype(NPFP8)
    # T8[fo, p, c2, t, n] = T[c2*256 + t*128 + p, fo*128 + n]
    T8 = np.ascontiguousarray(
        T8f.reshape(2, 2, 128, FO, 128).transpose(3, 2, 0, 1, 4)
    )

    p = np.arange(128)
    r = np.arange(32)
    # ones8dr[v, p, t, r] = 1 iff r == 16*v + t*8 + p//16
    t_ = np.arange(2)
    r128 = np.arange(128)
    on8 = (r128[None, None, None, :]
           == 16 * np.arange(8)[:, None, None, None]
           + 8 * t_[None, None, :, None]
           + (p[None, :, None, None] // 16)).astype(NPFP8)
    on8 = np.ascontiguousarray(on8.reshape(8, 128, 256))
    # ones16[q, p, r] = 1 iff r == q*8 + p//16
    on16 = (r[None, None, :] == 8 * np.arange(4)[:, None, None]
            + (p[None, :, None] // 16)).astype(NPBF16)
    # onesA[q, p, r] = 1 iff r == q*8 + p%8
    onA = (r[None, None, :] == 8 * np.arange(4)[:, None, None]
           + (p[None, :, None] % 8)).astype(NPBF16)
    on16 = np.ascontiguousarray(on16)
    onA = np.ascontiguousarray(onA)

    in_maps = []
    for c in range(N_CORES):
        rows = np.concatenate(
            [np.arange(s * STRIP, (s + 1) * STRIP) for s in SLOTS[c]]
        )
        xp = x8[rows]                          # [288, 512]
        # xT8[p, c2, t, col] = xp[col, c2*256 + t*128 + p]
        xT8 = np.ascontiguousarray(
            xp.T.reshape(2, 2, 128, NCOL).transpose(2, 0, 1, 3)
        )
        in_maps.append({
            "xT8": xT8,
            "T8": T8,
            "ones8dr": on8,
            "ones16": on16,
            "onesA": onA,
        })
    return in_maps


def assemble(x: np.ndarray, results) -> np.ndarray:
    out_pair = np.zeros((B, OUT_F), np.float32)
    for c in range(N_CORES):
        s = SLOTS[c]
        oi = results[c]["out_i"].astype(np.float32)   # [128, 2, 256]
        oj = results[c]["out_j"].astype(np.float32)   # [128, 17, 16]
        for u in range(NUNIT):
            sv, t = s[u], s[u + 1]
            bw, w, rs = u // 16, (u % 16) // 4, u % 4
            bi = oi[32 * w + 8 * rs:32 * w + 8 * rs + 8, bw]  # [8, 256]
            bi = bi.reshape(8, STRIP, FO)                     # [f8, j, fo]
            out_pair[t * STRIP:(t + 1) * STRIP] += (
                bi.transpose(1, 2, 0).reshape(STRIP, OUT_F)
            )
            if sv != t:
                bj = oj[:, u, :].reshape(STRIP, 8, FO)        # [i, f8, fo]
                out_pair[sv * STRIP:(sv + 1) * STRIP] += (
                    bj.transpose(0, 2, 1).reshape(STRIP, OUT_F)
                )
    out_pair -= 1.0
    out = np.empty((B, IN_F + OUT_F), np.float32)
    out[:, :IN_F] = x
    out[:, IN_F:] = out_pair
    return out


_NC_CACHE = None


def kernel(x: np.ndarray, T: np.ndarray) -> np.ndarray:
    global _NC_CACHE
    from concourse import bass_utils

    if _NC_CACHE is None:
        _NC_CACHE = build_nc()
    nc = _NC_CACHE
    in_maps = make_in_maps(np.asarray(x, np.float32), np.asarray(T, np.float32))
    res = bass_utils.run_bass_kernel_spmd(nc, in_maps, core_ids=list(range(N_CORES)))
    return assemble(np.asarray(x, np.float32), res.results)


# revision 14
# speedup vs baseline: 1.3766x; 1.0033x over previous
"""MiniBatchDiscrimination Trainium2 kernel (8-core SPMD, symmetric pairs).

Reference computation:
    m = (x @ T).reshape(B, OUT_F, NUM_K)            # B=256, OUT_F=128, NUM_K=16
    dists = |m[None,:,:,:] - m[:,None,:,:]|         # [B, B, OUT_F, NUM_K]
    out = sum_i exp(-sum_k dists) - 1               # [B, OUT_F]
    return concat([x, out], axis=-1)                # [B, 640]

Strategy:
  * The pairwise matrix is symmetric.  Partition the 256 rows into 16
    strips of 16; the 136 unordered strip pairs are covered by 8
    edge-disjoint trails of 17 edges (Walecki Hamiltonian paths of K16 plus
    one loop per vertex, two loops inserted per path).  Core c walks trail
    c: 18 strip slots, 17 edges.  Consecutive edge pairs (2d, 2d+1) share
    slot 2d+1; processing edge 2d transposed makes slot 2d+1 the common
    i-side, so each "double unit" d computes one 16i x 32j block
    (i = slot[2d+1], j = slots[2d] ++ slots[2d+2]) with 512-column
    matmuls; edge 16 is a single 16x16 unit.  Every ordered pair is
    computed exactly once globally; the host adds the partial sums.
  * GEMM: x/T in fp8e4, DoubleRow matmuls.  m bf16 [p=(f8,k), col, fo],
    f = fo*8+f8, p = f8*16+k, 288 cols = 18 slots x 16 rows.
  * Per double unit: DVE bf16 tensor_sub (2x mode, split in 4 so early
    units overlap the GEMM) -> abs split between ACT (Abs, fp8 out),
    GpSimd (abs_max, fp8 out) and DVE (sign-strip, bf16) -> k-sum on
    TensorE: fp8 DoubleRow matmuls (2 i's each, full-width weights since
    DoubleRow requires dst partition 0) + plain bf16 matmuls in 64-row
    windows -> ACT Exp(scale=-1) -> bf16 et -> i-sum matmul into batched
    PSUM windows + j-sums via DVE tensor_reduce over the free j axis.
  * Host: scatter-add the i/j-sum partials per the trail map, -1, concat x.
"""

import os
import numpy as np

import concourse.bass as bass
import concourse.tile as tile
from concourse import bacc, mybir

BF16 = mybir.dt.bfloat16
FP32 = mybir.dt.float32
FP8 = mybir.dt.float8e4
U16 = mybir.dt.uint16
NPBF16 = np.dtype(mybir.dt.np(BF16))
NPFP8 = np.dtype(mybir.dt.np(FP8))
DR = mybir.MatmulPerfMode.DoubleRow

B = 256
IN_F = 512
OUT_F = 128
NUM_K = 16
N_CORES = 8
STRIP = 16
NSLOT = 18
NUNIT = 17
NDU = 8                    # double units; edge 16 handled as a single
NCOL = NSLOT * STRIP       # 288
F8 = 8
FO = OUT_F // F8           # 16

# abs split over the 16 i's of each (double) unit: ACT -> fp8, GpSimd ->
# fp8, DVE sign-strip -> bf16.  N_ACT + N_GP must be even (DoubleRow pairs).
N_ACT = int(os.environ.get("N_ACT", "10"))
N_GP = int(os.environ.get("N_GP", "0"))
N_DVE = 16 - N_ACT - N_GP
N_FP8 = N_ACT + N_GP
assert N_FP8 % 2 == 0 and N_DVE >= 0 and N_FP8 > 0

SLOTS = [
    [0, 0, 1, 1, 15, 2, 14, 3, 13, 4, 12, 5, 11, 6, 10, 7, 9, 8],
    [1, 2, 2, 0, 3, 3, 15, 4, 14, 5, 13, 6, 12, 7, 11, 8, 10, 9],
    [2, 3, 1, 4, 4, 0, 5, 5, 15, 6, 14, 7, 13, 8, 12, 9, 11, 10],
    [3, 4, 2, 5, 1, 6, 6, 0, 7, 7, 15, 8, 14, 9, 13, 10, 12, 11],
    [4, 5, 3, 6, 2, 7, 1, 8, 8, 0, 9, 9, 15, 10, 14, 11, 13, 12],
    [5, 6, 4, 7, 3, 8, 2, 9, 1, 10, 10, 0, 11, 11, 15, 12, 14, 13],
    [6, 7, 5, 8, 4, 9, 3, 10, 2, 11, 1, 12, 12, 0, 13, 13, 15, 14],
    [7, 8, 6, 9, 5, 10, 4, 11, 3, 12, 2, 13, 1, 14, 14, 0, 15, 15],
]


def build_nc():
    nc = bacc.Bacc(name="minibatch_discrim_sym")

    xT_d = nc.dram_tensor("xT8", [128, 2, 2, NCOL], FP8, kind="ExternalInput")
    T_d = nc.dram_tensor("T8", [FO, 128, 2, 2, 128], FP8, kind="ExternalInput")
    # DoubleRow k-sum weights: ones8dr[v, p, t, r] = 1 iff r == 16*v + t*8 + p//16
    on8_d = nc.dram_tensor("ones8dr", [8, 128, 2 * 128], FP8, kind="ExternalInput")
    # plain k-sum weights: ones16[q, p, r] = 1 iff r == q*8 + p//16 (64-row windows)
    on16_d = nc.dram_tensor("ones16", [8, 128, 64], BF16, kind="ExternalInput")
    # i-sum weights: onesA[q, p, r] = 1 iff r == q*8 + p%8
    onA_d = nc.dram_tensor("onesA", [4, 128, 32], BF16, kind="ExternalInput")
    # outputs: i-sum partials (3 window slots in one PSUM bank) + j-sums
    outi_d = nc.dram_tensor("out_i", [128, 2 * STRIP * FO], FP32, kind="ExternalOutput")
    outj_d = nc.dram_tensor("out_j", [128, NUNIT, FO], FP32, kind="ExternalOutput")

    with tile.TileContext(nc) as tc:
        with (
            tc.tile_pool(name="const", bufs=1) as constp,
            tc.tile_pool(name="mm", bufs=1) as mmp,
            tc.tile_pool(name="gpsum", bufs=2, space=bass.MemorySpace.PSUM) as gps,
            tc.tile_pool(name="dpsum", bufs=3, space=bass.MemorySpace.PSUM) as dps,
            tc.tile_pool(name="apsum", bufs=1, space=bass.MemorySpace.PSUM) as aps,
            tc.tile_pool(name="work", bufs=3) as wp,
            tc.tile_pool(name="expp", bufs=3) as ep,
        ):
            # ---- constants / inputs to SBUF ----
            zero_b = constp.tile([128, 1], FP32)
            nc.gpsimd.memset(zero_b[:], 0.0)

            ones8 = constp.tile([128, 8, 2, 128], FP8)
            nc.sync.dma_start(ones8[:], on8_d.rearrange("v p (t r) -> p v t r", r=128))
            ones16 = constp.tile([128, 8, 64], BF16)
            nc.sync.dma_start(ones16[:], on16_d.rearrange("q p r -> p q r"))
            onesA = constp.tile([128, 4, 32], BF16)
            nc.sync.dma_start(onesA[:], onA_d.rearrange("q p r -> p q r"))

            # warm the ACT table while DMAs run
            warm = constp.tile([128, 1], FP32)
            nc.scalar.activation(
                warm[:], zero_b[:], mybir.ActivationFunctionType.Abs, bias=zero_b[:]
            )
            nc.scalar.activation(
                warm[:], zero_b[:], mybir.ActivationFunctionType.Exp, bias=zero_b[:]
            )

            xT_sb = constp.tile([128, 2, 2, NCOL], FP8)
            nc.sync.dma_start(xT_sb[:], xT_d[:])
            T_tiles = []
            for fo in range(FO):
                tt = constp.tile([128, 2, 2, 128], FP8, tag=f"T{fo}")
                nc.sync.dma_start(tt[:], T_d[fo])
                T_tiles.append(tt)

            # ---- GEMM (fp8 DoubleRow): m[p=(f8,k), col, fo] bf16 ----
            # copies on DVE: it is idle during the GEMM phase
            m_sb = mmp.tile([128, NCOL, FO], BF16)
            for fo in range(FO):
                pm = gps.tile([128, NCOL], FP32, tag="gemm")
                for c2 in range(2):
                    nc.tensor.matmul(
                        pm[:],
                        T_tiles[fo][:, c2],
                        xT_sb[:, c2],
                        start=(c2 == 0),
                        stop=(c2 == 1),
                        perf_mode=DR,
                    )
                if fo % 2 == 0:
                    nc.vector.tensor_copy(m_sb[:, :, fo], pm[:])
                else:
                    nc.scalar.copy(m_sb[:, :, fo], pm[:])

            # ---- unit loop: 8 double units + 1 single ----
            outj_sb = mmp.tile([128, NUNIT, FO], FP32)
            outi_ps = aps.tile([128, 2 * STRIP * FO], FP32)
            outi_sb = mmp.tile([128, 2 * STRIP * FO], FP32)

            def do_unit(d, jw, icol, jcols):
                """One (double) unit: i strip at m column icol, j strips at
                m columns jcols (len jw//16).  d indexes output slots."""
                nj = len(jcols)
                # diff[p, i, j, fo]; split per j-slot and fo-half so early
                # units only depend on early GEMM fo's
                diff = wp.tile([128, STRIP, jw, FO], BF16, tag="diff")
                for h2 in range(nj):
                    for fh in range(2):
                        nc.vector.tensor_sub(
                            diff[:, :, 16 * h2:16 * (h2 + 1), 8 * fh:8 * (fh + 1)],
                            m_sb[:, icol:icol + STRIP, None, 8 * fh:8 * (fh + 1)]
                            .broadcast_to([128, STRIP, 16, 8]),
                            m_sb[:, None, jcols[h2]:jcols[h2] + 16,
                                 8 * fh:8 * (fh + 1)]
                            .broadcast_to([128, STRIP, 16, 8]),
                        )

                ad8 = wp.tile([128, N_FP8, jw, FO], FP8, tag="ad8")
                if N_ACT > 0:
                    nc.scalar.activation(
                        ad8[:, :N_ACT], diff[:, :N_ACT],
                        mybir.ActivationFunctionType.Abs, bias=zero_b[:],
                    )
                if N_GP > 0:
                    nc.gpsimd.tensor_scalar(
                        ad8[:, N_ACT:N_FP8], diff[:, N_ACT:N_FP8],
                        0.0, None, op0=mybir.AluOpType.abs_max,
                    )
                if N_DVE > 0:
                    ad16 = wp.tile([128, N_DVE, jw, FO], BF16, tag="ad16")
                    nc.vector.tensor_scalar(
                        ad16[:].bitcast(U16),
                        diff[:, N_FP8:].bitcast(U16),
                        0x7FFF, None, op0=mybir.AluOpType.bitwise_and,
                    )

                # k-sum into pd[p=(i,f8), (j,fo)].  DoubleRow needs dst
                # partition 0 -> full-width per-pair weights; bf16 matmuls
                # use 64-row windows.
                pd = dps.tile([128, jw, FO], FP32, tag="dist")
                n_ops = N_FP8 // 2 + N_DVE
                k_ = 0
                for p2 in range(N_FP8 // 2):
                    nc.tensor.matmul(
                        pd[:], ones8[:, p2], ad8[:, 2 * p2:2 * p2 + 2],
                        start=(k_ == 0), stop=(k_ == n_ops - 1),
                        perf_mode=DR, tile_position=(0, 0),
                        skip_group_check=True,
                    )
                    k_ += 1
                for i0 in range(N_FP8, 16):
                    w64 = i0 // 8
                    nc.tensor.matmul(
                        pd[64 * w64:64 * (w64 + 1)],
                        ones16[:, i0 % 8], ad16[:, i0 - N_FP8],
                        start=(k_ == 0), stop=(k_ == n_ops - 1),
                        tile_position=(0, 64 * w64),
                        skip_group_check=True,
                    )
                    k_ += 1

                et = ep.tile([128, jw, FO], BF16, tag="expt")
                nc.scalar.activation(
                    et[:], pd[:],
                    mybir.ActivationFunctionType.Exp, bias=zero_b[:], scale=-1.0,
                )

                # i-sum: rows rs*8+f8 of 32-row window w32 in the shared bank
                w32, rs = d // 4, d % 4
                nc.tensor.matmul(
                    outi_ps[32 * w32:32 * (w32 + 1), :jw * FO],
                    onesA[:, rs],
                    et[:],
                    start=(rs == 0), stop=(rs == 3 or d == 8),
                    skip_group_check=True,
                    tile_position=(0, 32 * w32),
                )

                # j-sums per j-slot -> [p=(i,f8), nj, fo]
                nc.vector.tensor_reduce(
                    outj_sb[:, 2 * d:2 * d + nj, :],
                    et[:].rearrange("p (j2 j) f -> p j2 f j", j2=nj),
                    mybir.AxisListType.X,
                    mybir.AluOpType.add,
                )

            for d in range(NDU):
                # edges 2d (transposed) and 2d+1: i = slot 2d+1
                do_unit(d, 32, (2 * d + 1) * STRIP,
                        [2 * d * STRIP, (2 * d + 2) * STRIP])
            # final single edge 16: i = slot 17, j = slot 16
            do_unit(8, 16, 17 * STRIP, [16 * STRIP])

            nc.scalar.copy(outi_sb[:96], outi_ps[:96])
            nc.sync.dma_start(outi_d[:], outi_sb[:])
            nc.sync.dma_start(outj_d[:], outj_sb[:])

    nc.finalize()
    return nc


def make_in_maps(x: np.ndarray, T: np.ndarray):
    x8 = x.astype(NPFP8)
    T8f = T.astype(NPFP8)
    T8 = np.ascontiguousarray(
        T8f.reshape(2, 2, 128, FO, 128).transpose(3, 2, 0, 1, 4)
    )

    p = np.arange(128)
    t_ = np.arange(2)
    r128 = np.arange(128)
    on8 = (r128[None, None, None, :]
           == 16 * np.arange(8)[:, None, None, None]
           + 8 * t_[None, None, :, None]
           + (p[None, :, None, None] // 16)).astype(NPFP8)
    on8 = np.ascontiguousarray(on8.reshape(8, 128, 256))
    r64 = np.arange(64)
    on16 = (r64[None, None, :] == 8 * np.arange(8)[:, None, None]
            + (p[None, :, None] // 16)).astype(NPBF16)
    r32 = np.arange(32)
    onA = (r32[None, None, :] == 8 * np.arange(4)[:, None, None]
           + (p[None, :, None] % 8)).astype(NPBF16)
    on16 = np.ascontiguousarray(on16)
    onA = np.ascontiguousarray(onA)

    in_maps = []
    for c in range(N_CORES):
        rows = np.concatenate(
            [np.arange(s * STRIP, (s + 1) * STRIP) for s in SLOTS[c]]
        )
        xp = x8[rows]
        xT8 = np.ascontiguousarray(
            xp.T.reshape(2, 2, 128, NCOL).transpose(2, 0, 1, 3)
        )
        in_maps.append({
            "xT8": xT8,
            "T8": T8,
            "ones8dr": on8,
            "ones16": on16,
            "onesA": onA,
        })
    return in_maps


def assemble(x: np.ndarray, results) -> np.ndarray:
    out_pair = np.zeros((B, OUT_F), np.float32)
    for c in range(N_CORES):
        s = SLOTS[c]
        oi = results[c]["out_i"].astype(np.float32)   # [128, 1024]
        oj = results[c]["out_j"].astype(np.float32)   # [128, 17, 16]
        # double unit d: i strip s[2d+1], j strips (s[2d], s[2d+2])
        # single (d=8): i strip s[17], j strip s[16]
        for d in range(9):
            if d < 8:
                si, js = s[2 * d + 1], [s[2 * d], s[2 * d + 2]]
            else:
                si, js = s[17], [s[16]]
            w32, rs = d // 4, d % 4
            jw = 16 * len(js)
            bi = oi[32 * w32 + 8 * rs:32 * w32 + 8 * rs + 8, :jw * FO]
            bi = bi.reshape(8, len(js), STRIP, FO)    # [f8, j2, j, fo]
            bj = oj[:, 2 * d:2 * d + len(js), :].reshape(STRIP, 8, len(js), FO)
            for h2, t in enumerate(js):
                # i-sum partial -> rows of strip t
                out_pair[t * STRIP:(t + 1) * STRIP] += (
                    bi[:, h2].transpose(1, 2, 0).reshape(STRIP, OUT_F)
                )
                # j-sum partial -> rows of strip si (skip if diagonal block)
                if t != si:
                    out_pair[si * STRIP:(si + 1) * STRIP] += (
                        bj[:, :, h2].transpose(0, 2, 1).reshape(STRIP, OUT_F)
                    )
    out_pair -= 1.0
    out = np.empty((B, IN_F + OUT_F), np.float32)
    out[:, :IN_F] = x
    out[:, IN_F:] = out_pair
    return out


_NC_CACHE = None


def kernel(x: np.ndarray, T: np.ndarray) -> np.ndarray:
    global _NC_CACHE
    from concourse import bass_utils

    if _NC_CACHE is None:
        _NC_CACHE = build_nc()
    nc = _NC_CACHE
    in_maps = make_in_maps(np.asarray(x, np.float32), np.asarray(T, np.float32))
    res = bass_utils.run_bass_kernel_spmd(nc, in_maps, core_ids=list(range(N_CORES)))
    return assemble(np.asarray(x, np.float32), res.results)
